# revision 18
# baseline (speedup 1.0000x reference)
"""Batched matrix-attention scores kernel for Trainium2 (8 NeuronCores).

Computes scores[b, i, j] = sum_d m1[b, i, d] * m2[b, j, d]
  (i.e. jnp.einsum('bid,bjd->bij', matrix_1, matrix_2))
with B=16, R1=R2=2048, D=256, fp32 in/out.

Sharding: data-parallel over batch — 2 batches per core on 8 cores.

Per-core structure:
  - Operands are PE-transposed (matmul with identity) into
    D-on-partitions layout mT[d, dc, row] since the tensor engine
    contracts over the partition dim; fp32 can't use DMA transpose.
  - Each 128-row output tile takes 8 matmuls (2 d-chunks x 4 j-chunks
    of N=512, one fp32 PSUM bank each); PSUM is evacuated on DVE+ACT,
    output stored in 2MB blocks on the Sync DMA ring.
  - Batch b+1's loads (Scalar DMA ring) and transposes are interleaved
    into batch b's matmul phase so the store pipe never drains.

Operands use dt.float32r (fp32 bits, full-rate single-pass PE matmul;
~2^-11 input mantissa truncation) — ~4x the fp32 matmul rate.
Accumulation stays fp32 in PSUM.
"""

from contextlib import ExitStack

import numpy as np

import concourse.bass as bass
import concourse.mybir as mybir
import concourse.tile as tile
from concourse import bacc
from concourse.bass_utils import run_bass_kernel_spmd

F32 = mybir.dt.float32
F32R = mybir.dt.float32r

NCORES = 8
B, R1, R2, D = 16, 2048, 2048, 256
BPC = B // NCORES  # batches per core
P = 128
NJ_TILE = 512  # matmul free dim (one fp32 PSUM bank)
NJ = R2 // NJ_TILE  # j-chunks per row-block
NT = R1 // P  # 128-row tiles per batch
DC = D // P  # contraction chunks


def _build_tile_kernel(ctx: ExitStack, tc: tile.TileContext, m1, m2, ident_in, out):
    nc = tc.nc

    const_pool = ctx.enter_context(tc.tile_pool(name="const", bufs=1))
    ident = const_pool.tile([P, P], F32R)
    nc.scalar.dma_start(ident, ident_in)

    nat_pool = ctx.enter_context(tc.tile_pool(name="nat", bufs=2 * BPC))
    mt_pool = ctx.enter_context(tc.tile_pool(name="mt", bufs=2))
    tpsum = ctx.enter_context(tc.tile_pool(name="tpsum", bufs=2, space="PSUM"))
    mpsum = ctx.enter_context(tc.tile_pool(name="mpsum", bufs=6, space="PSUM"))
    outp = ctx.enter_context(tc.tile_pool(name="outp", bufs=6))

    def emit_loads(b, first):
        nats = []
        for name, src in (("m2", m2), ("m1", m1)):
            nat = nat_pool.tile([P, NT, D], F32R, tag="nat", name=f"nat_{name}_{b}")
            nchunk = (16 if name == "m2" else 4) if first else 2
            ostep = NT // nchunk
            dma_eng = nc.sync if name == "m2" else nc.scalar
            for c in range(nchunk):
                dma_eng.dma_start(
                    nat[:, c * ostep : (c + 1) * ostep, :],
                    src[b].rearrange("(o p) d -> p o d", p=P)[
                        :, c * ostep : (c + 1) * ostep, :
                    ],
                )
            nats.append(nat)
        return nats

    def alloc_mts(b):
        return [
            mt_pool.tile([P, DC, R1], F32R, tag=name, name=f"{name}_{b}")
            for name in ("m2T", "m1T")
        ]

    def emit_transpose_unit(b, mi, nats, mts, o, start_eng):
        """Two PE transposes (dc=0,1) of one 128-row block, casts on
        alternating engines."""
        for dc in range(DC):
            ps = tpsum.tile([P, P], F32R, tag="tps", name=f"tps_{b}_{mi}_{o}_{dc}")
            nc.tensor.transpose(ps, nats[mi][:, o, dc * P : (dc + 1) * P], ident)
            dst = mts[mi][:, dc, o * P : (o + 1) * P]
            if (dc + start_eng) % 2 == 0:
                nc.vector.tensor_copy(dst, ps)
            else:
                nc.scalar.copy(dst, ps)

    def emit_mm_block(b, mts, it2, act_heavy=False):
        m2T, m1T = mts
        for half in range(2):
            it = it2 * 2 + half
            stage = outp.tile([P, R2], F32, tag="stage", name=f"stage_{b}_{it}")
            pss = [
                mpsum.tile([P, NJ_TILE], F32, tag="mps", name=f"mps_{b}_{it}_{jc}")
                for jc in range(NJ)
            ]
            for dc in range(DC):
                for jc in range(NJ):
                    nc.tensor.matmul(
                        pss[jc],
                        m1T[:, dc, it * P : (it + 1) * P],
                        m2T[:, dc, jc * NJ_TILE : (jc + 1) * NJ_TILE],
                        start=(dc == 0),
                        stop=(dc == DC - 1),
                    )
            for jc in range(NJ):
                dst = stage[:, jc * NJ_TILE : (jc + 1) * NJ_TILE]
                use_dve = (jc == 0) if act_heavy else (jc % 2 == 0)
                if use_dve:
                    nc.vector.tensor_copy(dst, pss[jc])
                else:
                    nc.scalar.copy(dst, pss[jc])
            nc.sync.dma_start(out[b, it * P : (it + 1) * P, :], stage)

    # all input loads issued up front (m2 on the sync ring, m1 on the
    # scalar ring) — no deps, so they pipeline ahead of the stores
    all_nats = [emit_loads(b, first=(b == 0)) for b in range(BPC)]
    all_mts = [alloc_mts(b) for b in range(BPC)]

    # batch-0 transposes upfront, alternating matrices so the PE can chew
    # m1 blocks (scalar-ring loads) while m2 chunks (sync ring) arrive
    eng = 0
    for o in range(NT):
        for mi in (0, 1):
            emit_transpose_unit(0, mi, all_nats[0], all_mts[0], o, eng)
            eng += 1

    for b in range(BPC):
        for it2 in range(NT // 2):
            emit_mm_block(b, all_mts[b], it2, act_heavy=(b + 1 < BPC and it2 < 4))
            # front-load next batch's transposes into the first 4 blocks,
            # while the DMA engines are still busy with input loads
            if b + 1 < BPC and it2 < 4:
                for o in range(4 * it2, 4 * it2 + 4):
                    for mi in (0, 1):
                        emit_transpose_unit(
                            b + 1, mi, all_nats[b + 1], all_mts[b + 1], o, eng
                        )
                        eng += 1


_NC_CACHE = None


def _build():
    global _NC_CACHE
    if _NC_CACHE is not None:
        return _NC_CACHE
    nc = bacc.Bacc(
        "TRN2", target_bir_lowering=False, debug=False, num_devices=NCORES
    )
    m1 = nc.dram_tensor("m1", [BPC, R1, D], F32R, kind="ExternalInput").ap()
    m2 = nc.dram_tensor("m2", [BPC, R2, D], F32R, kind="ExternalInput").ap()
    ident_in = nc.dram_tensor("ident", [P, P], F32R, kind="ExternalInput").ap()
    out = nc.dram_tensor("out", [BPC, R1, R2], F32, kind="ExternalOutput").ap()
    with tile.TileContext(nc) as tc:
        with ExitStack() as ctx:
            _build_tile_kernel(ctx, tc, m1, m2, ident_in, out)
    nc.compile()
    _NC_CACHE = nc
    return nc


def kernel(matrix_1: np.ndarray, matrix_2: np.ndarray, **run_kwargs) -> np.ndarray:
    m1 = np.ascontiguousarray(np.asarray(matrix_1, dtype=np.float32))
    m2 = np.ascontiguousarray(np.asarray(matrix_2, dtype=np.float32))
    assert m1.shape == (B, R1, D) and m2.shape == (B, R2, D)

    nc = _build()
    eye = np.eye(P, dtype=np.float32)
    in_maps = [
        {
            "m1": m1[i * BPC : (i + 1) * BPC],
            "m2": m2[i * BPC : (i + 1) * BPC],
            "ident": eye,
        }
        for i in range(NCORES)
    ]
    res = run_bass_kernel_spmd(
        nc, in_maps, core_ids=list(range(NCORES)), **run_kwargs
    )
    out = np.empty((B, R1, R2), dtype=np.float32)
    for i in range(NCORES):
        out[i * BPC : (i + 1) * BPC] = res.results[i]["out"]
    if run_kwargs:
        kernel.last_result = res
    return out
